# revision 38
# baseline (speedup 1.0000x reference)
"""Trainium2 Bass kernel for nn_Decoder (mlp3 + segment_sum decoder), 8 cores.

Strategy (data-parallel over nodes N, segment-aligned shard boundaries):
  - Host: shard rows so core c owns whole segments [128c, 128(c+1)); transpose
    x to [H, rows] fp16 so stage-1 matmul lhsT streams straight from HBM;
    precompute folded biases (the ssp "-ln2" shift folded into the next
    stage's bias / the final projection's per-segment count correction).
  - Device (per core, SPMD), per 128-row subtile j, per stage s:
      matmul (fp16, weights moving) -> z in PSUM f32;
      variance via tensor_tensor_reduce (z*z/H + eps in one DVE instr/pass);
      rstd = exp(-0.5*ln(var+eps)) on ACT (tiny, one act-table set);
      normalize+exp: either ACT Exp(scale=rstd) x5 (route U) or the
      normalize-mult offloaded to the idle Pool/GPSIMD engine (route P) or
      DVE (route D) followed by ONE batched ACT Exp;
      stages 0/1: PE transpose (fp16 PSUM out) then ONE batched ACT
      Ln(1+x) -> fp16 SBUF for the next stage's lhsT;
      stage 2: batched Ln then one-hot segment matmul accumulating pooled
      sums in PSUM (proc batched [P,512] + enc tail in the same acc tile).
  - Final tiny projection (pooled @ wp/we + consts) on device; host gathers
    the per-core [128 segs, T] outputs into [T, G].
"""
import sys
sys.path.insert(0, "/opt/trn_rl_repo")
import numpy as np

T, N, H, G = 4, 100000, 128, 1024
NCORES = 8
SEG_PER_CORE = G // NCORES        # 128
LN2 = float(np.log(2.0))
EPS = 1e-5
P = 128

# tunable build config
CFG = {
    "BJ": 16,       # j's interleaved per pipeline block
    "JG": 8,        # subtiles loaded per DMA group (one dma_start each)
    "skew": (2, 5),  # software pipeline depth (B1, B2)
    # per-(j,s) normalize route: "U" ACT-scaled Exp x5 (cheap on DVE);
    # "D" DVE normalize + one batched ACT Exp (cheap on ACT).
    "route": lambda j, s: "U",
    "zp_bufs": 4,   # PSUM z4 double-buffer depth
    "et_bufs": 1,   # PSUM transpose-out buffer depth
    "rstd": "act",  # "act" = exp(-.5 ln v) on ACT (pair-batched); "quake" = DVE rsqrt
}

_cache = {}


def _blob_layout(nsub):
    """Offsets (in elements) of each logical tensor inside the two input
    blobs. Packing everything into 2 DRAM buffers cuts per-run dispatch
    overhead (~50us per buffer per run on the PJRT/axon path)."""
    nloc = nsub * P
    l16, o = {}, 0
    for name, shape in (("xt5", (H, 5, nloc)), ("s16d", (P, nloc)),
                        ("pw16", (3, H, H)), ("ew16", (3, H, H)),
                        ("pb16", (3, H)), ("eb16", (3, H)),
                        ("ident", (H, H))):
        l16[name] = (o, shape)
        o += int(np.prod(shape))
    n16 = o
    l32, o = {}, 0
    for name, shape in (("wp4b", (P, T * H)), ("web", (P, H)),
                        ("kvec", (P, 1))):
        l32[name] = (o, shape)
        o += int(np.prod(shape))
    return l16, n16, l32, o


_tables_patched = False


def _patch_act_tables():
    """Make the table chooser put Exp and Ln in one set (one load total)."""
    global _tables_patched
    if _tables_patched:
        return
    import concourse.bacc as bacc
    from concourse import mybir, hw_specs
    _orig = hw_specs.get_activation_tables

    def patched(arch):
        t = _orig(arch)
        AF = mybir.ActivationFunctionType
        for name, funcs in t.items():
            if name != "natural_log_exp_and_others":
                t[name] = {f for f in funcs if f not in (AF.Exp, AF.Ln)}
        return t

    bacc.get_activation_tables = patched
    _tables_patched = True


def _build(nsub, use_bias0p, use_bias0e, cfg=None):
    import concourse.bass as bass
    import concourse.bacc as bacc
    import concourse.tile as tile
    from concourse import mybir
    _patch_act_tables()
    cfg = dict(CFG, **(cfg or {}))
    route = cfg["route"]
    F16, F32 = mybir.dt.float16, mybir.dt.float32
    I32 = mybir.dt.int32
    AF = mybir.ActivationFunctionType
    OP = mybir.AluOpType
    MAGIC = 0x5f375a86

    BJ = cfg["BJ"]
    JG = cfg["JG"]
    SKEW = cfg["skew"]
    assert BJ % JG == 0
    blocks = []
    jb0 = 0
    while jb0 < nsub:
        blocks.append((jb0, min(BJ, nsub - jb0)))
        jb0 += BJ
    nc = bacc.Bacc("TRN2", target_bir_lowering=False, debug=False,
                   enable_asserts=True, num_devices=NCORES)

    l16, n16, l32, n32 = _blob_layout(nsub)
    blob16 = nc.dram_tensor("b16", [n16], F16, kind="ExternalInput").ap()
    blob32 = nc.dram_tensor("b32", [n32], F32, kind="ExternalInput").ap()
    res = nc.dram_tensor("res", [P, T], F32, kind="ExternalOutput").ap()

    def _sub(blob, layout, name):
        off, shape = layout[name]
        ap = [[int(np.prod(shape[i + 1:])), shape[i]] for i in range(len(shape))]
        return bass.AP(tensor=blob.tensor, offset=blob.offset + off, ap=ap)

    xt5 = _sub(blob16, l16, "xt5")
    s16d = _sub(blob16, l16, "s16d")
    pw16 = _sub(blob16, l16, "pw16")
    ew16 = _sub(blob16, l16, "ew16")
    pb16 = _sub(blob16, l16, "pb16")
    eb16 = _sub(blob16, l16, "eb16")
    ident = _sub(blob16, l16, "ident")
    wp4b = _sub(blob32, l32, "wp4b")
    web = _sub(blob32, l32, "web")
    kvec = _sub(blob32, l32, "kvec")

    with tile.TileContext(nc) as tc:
        import contextlib
        with contextlib.ExitStack() as ctx:
            singles = ctx.enter_context(tc.tile_pool(name="singles", bufs=1))
            xload = ctx.enter_context(tc.tile_pool(name="xload", bufs=4))
            sload = ctx.enter_context(tc.tile_pool(name="sload", bufs=3))
            work = ctx.enter_context(tc.tile_pool(name="work", bufs=6))
            scrp = ctx.enter_context(tc.tile_pool(name="scr", bufs=2))
            blockp = ctx.enter_context(tc.tile_pool(name="blockp", bufs=12))
            stat = ctx.enter_context(tc.tile_pool(name="stat", bufs=16))
            zpool = ctx.enter_context(tc.tile_pool(name="zp", bufs=cfg["zp_bufs"], space="PSUM"))
            etpool = ctx.enter_context(tc.tile_pool(name="etp", bufs=cfg["et_bufs"], space="PSUM"))
            mis = ctx.enter_context(tc.tile_pool(name="mis", bufs=2, space="PSUM"))
            acc = ctx.enter_context(tc.tile_pool(name="acc", bufs=1, space="PSUM"))

            # --- one-time constants ---
            w16 = []
            for fam, src in (("p", pw16), ("e", ew16)):
                fam_tiles = []
                for si in range(3):
                    wt = singles.tile([H, H], F16, tag=f"w{fam}{si}")
                    nc.sync.dma_start(out=wt, in_=src[si])
                    fam_tiles.append(wt)
                w16.append(fam_tiles)
            b16 = []
            for fam, src in (("p", pb16), ("e", eb16)):
                fam_tiles = []
                for si in range(3):
                    bt = singles.tile([1, H], F16, tag=f"b{fam}{si}")
                    nc.sync.dma_start(out=bt, in_=src[si:si + 1, :])
                    fam_tiles.append(bt)
                b16.append(fam_tiles)
            i16 = singles.tile([H, H], F16, tag="ident")
            nc.sync.dma_start(out=i16, in_=ident)
            ones16 = singles.tile([1, H], F16, tag="ones")
            nc.vector.memset(ones16, 1.0)
            eps_t = singles.tile([P, 1], F32, tag="eps")
            nc.vector.memset(eps_t, EPS)
            wp4t = singles.tile([P, T * H], F32, tag="wp4")
            nc.sync.dma_start(out=wp4t, in_=wp4b)
            webt = singles.tile([P, H], F32, tag="web")
            nc.sync.dma_start(out=webt, in_=web)
            kvt = singles.tile([P, 1], F32, tag="kv")
            nc.sync.dma_start(out=kvt, in_=kvec)

            # --- pooled accumulators: proc in PSUM, enc partials added into SBUF
            pp = acc.tile([P, T, H], F32, tag="pp")
            pp_proc = bass.AP(tensor=pp.tensor, offset=pp.offset,
                              ap=[pp.ap[0], [1, T * H]])
            pe_sb = singles.tile([P, H], F32, tag="pe_sb")
            nc.vector.memset(pe_sb, 0.0)

            use_bias = [[use_bias0p, True, True], [use_bias0e, True, True]]

            rstate = {}   # j -> (zs, v5)

            def emit_A(j, s, lhs, aggpairs, pi, sl):
                z4 = zpool.tile([P, T * H], F32, tag="z4")
                zm = mis.tile([P, 2, H], F32, tag="zm")
                zs = [z4[:, p * H:(p + 1) * H] for p in range(T)] + [zm[:, 0, :]]
                for p in range(5):
                    fam = 0 if p < T else 1
                    has_b = use_bias[fam][s]
                    nc.tensor.matmul(zs[p], lhsT=lhs[p], rhs=w16[fam][s],
                                     start=True, stop=not has_b)
                    if has_b:
                        nc.tensor.matmul(zs[p], lhsT=ones16, rhs=b16[fam][s],
                                         start=False, stop=True)
                stc = stat.tile([P, 5, 6], F32, tag="stc6")
                if sl == 0:
                    aggp = stat.tile([P, 2, 5, 2], F32, tag="aggp")
                    aggpairs[pi] = aggp
                agg = aggpairs[pi]
                for p in range(5):
                    nc.vector.bn_stats(out=stc[:, p, :], in_=zs[p])
                    nc.vector.bn_aggr(out=agg[:, sl, p, :], in_=stc[:, p, :])
                r5A = None
                if cfg["rstd"] == "quakeA":
                    # quake rsqrt emitted right here: stays DVE-local, so the
                    # ACT Exp in B1 never waits on a cross-engine rstd chain
                    agg1 = bass.AP(tensor=agg.tensor, offset=agg.offset + sl * 10,
                                   ap=[agg.ap[0], [2, 5], [1, 2]])
                    r5A = emit_rstd_quake_ap(agg1)
                return zs, zm, r5A

            def emit_rstd_quake_ap(agg1):
                """DVE bit-hack rsqrt of (var + eps); agg1 = [P,5,2] AP."""
                nv = 5
                var = bass.AP(tensor=agg1.tensor, offset=agg1.offset + 1,
                              ap=[agg1.ap[0], [2, nv]])
                return _quake_chain(var, nv)

            def emit_rstd_quake(agg, npair):
                """DVE bit-hack rsqrt of (var + eps) for `npair` j's at once."""
                nv = 5 * npair
                var = bass.AP(tensor=agg.tensor, offset=agg.offset + 1,
                              ap=[agg.ap[0], [2, nv]])
                return _quake_chain(var, nv)

            def _quake_chain(var, nv):
                vpe = stat.tile([P, nv], F32, tag="vpe")
                nc.vector.tensor_scalar(out=vpe, in0=var, scalar1=EPS,
                                        scalar2=None, op0=OP.add)
                t1 = stat.tile([P, nv], I32, tag="qt1")
                nc.vector.tensor_scalar(out=t1, in0=vpe.bitcast(I32),
                                        scalar1=1, scalar2=None,
                                        op0=OP.logical_shift_right)
                t1n = stat.tile([P, nv], I32, tag="qt1n")
                nc.vector.tensor_scalar(out=t1n, in0=t1, scalar1=0,
                                        scalar2=None, op0=OP.bitwise_not)
                t2 = stat.tile([P, nv], I32, tag="qt2")
                nc.vector.tensor_scalar(out=t2, in0=t1n, scalar1=MAGIC + 1,
                                        scalar2=None, op0=OP.add)
                y = t2.bitcast(F32)
                for it in range(2):
                    a = stat.tile([P, nv], F32, tag=f"qa{it}")
                    nc.vector.tensor_tensor(out=a, in0=y, in1=y, op=OP.mult)
                    b = stat.tile([P, nv], F32, tag=f"qb{it}")
                    nc.vector.scalar_tensor_tensor(out=b, in0=a, scalar=-0.5,
                                                   in1=vpe, op0=OP.mult,
                                                   op1=OP.mult)
                    y2 = stat.tile([P, nv], F32, tag=f"qy{it}")
                    nc.vector.scalar_tensor_tensor(out=y2, in0=b, scalar=1.5,
                                                   in1=y, op0=OP.add,
                                                   op1=OP.mult)
                    y = y2
                return y

            def emit_B1(j, s, state, aggpairs, rquake, pi, sl, npair):
                zs, zm, r5A = state
                agg = aggpairs[pi]
                if cfg["rstd"] == "quakeA":
                    r5 = r5A
                elif cfg["rstd"] == "quake":
                    if pi not in rquake:
                        rquake[pi] = emit_rstd_quake(agg, npair)
                    r5 = rquake[pi][:, sl * 5: sl * 5 + 5]
                elif cfg["rstd"] == "act2":
                    # rstd = exp(-0.5 * ln(var + eps)) on ACT, batched per pair
                    if pi not in rquake:
                        nv = 5 * npair
                        var = bass.AP(tensor=agg.tensor, offset=agg.offset + 1,
                                      ap=[agg.ap[0], [2, nv]])
                        l10 = stat.tile([P, nv], F32, tag="l10")
                        nc.scalar.activation(out=l10, in_=var, func=AF.Ln,
                                             bias=eps_t, scale=1.0)
                        r10 = stat.tile([P, nv], F32, tag="r10")
                        nc.scalar.activation(out=r10, in_=l10, func=AF.Exp,
                                             bias=0.0, scale=-0.5)
                        rquake[pi] = r10
                    r5 = rquake[pi][:, sl * 5: sl * 5 + 5]
                else:
                    # per-js rstd on ACT (lowest-latency chain)
                    var5 = bass.AP(tensor=agg.tensor,
                                   offset=agg.offset + sl * 10 + 1,
                                   ap=[agg.ap[0], [2, 5]])
                    l5 = stat.tile([P, 5], F32, tag="l5")
                    nc.scalar.activation(out=l5, in_=var5, func=AF.Ln,
                                         bias=eps_t, scale=1.0)
                    r5 = stat.tile([P, 5], F32, tag="r5")
                    nc.scalar.activation(out=r5, in_=l5, func=AF.Exp,
                                         bias=0.0, scale=-0.5)
                mode = route(j, s)
                e5 = work.tile([P, 5, H], F16, tag="e5")
                if mode == "U":
                    for p in range(5):
                        nc.scalar.activation(out=e5[:, p, :], in_=zs[p],
                                             func=AF.Exp, bias=0.0,
                                             scale=r5[:, p:p + 1])
                else:
                    zn5 = work.tile([P, 5, H], F16, tag="zn5")
                    for p in range(5):
                        nc.vector.tensor_scalar(
                            out=zn5[:, p, :], in0=zs[p],
                            scalar1=r5[:, p:p + 1], scalar2=None,
                            op0=OP.mult)
                    nc.scalar.activation(out=e5, in_=zn5, func=AF.Exp,
                                         bias=0.0, scale=1.0)
                return e5, zm

            def emit_B2(j, s, state, s16s):
                e5, zm = state
                if s < 2:
                    et5 = etpool.tile([P, 5, H], F16, tag="et5")
                    for p in range(5):
                        nc.tensor.transpose(et5[:, p, :], e5[:, p, :], i16)
                    at5 = blockp.tile([P, 5, H], F16, tag="at5")
                    nc.scalar.activation(out=at5, in_=et5, func=AF.Ln,
                                         bias=1.0, scale=1.0)
                    return [at5[:, p, :] for p in range(5)]
                else:
                    a5 = work.tile([P, 5, H], F16, tag="a5")
                    nc.scalar.activation(out=a5, in_=e5, func=AF.Ln,
                                         bias=1.0, scale=1.0)
                    a4_flat = bass.AP(tensor=a5.tensor, offset=a5.offset,
                                      ap=[a5.ap[0], [1, T * H]])
                    nc.tensor.matmul(pp_proc, lhsT=s16s, rhs=a4_flat,
                                     start=(j == 0), stop=(j == nsub - 1))
                    # enc pooled partial -> spare zm slot -> SBUF accumulator
                    nc.tensor.matmul(zm[:, 1, :], lhsT=s16s, rhs=a5[:, T, :],
                                     start=True, stop=True)
                    nc.vector.tensor_tensor(out=pe_sb, in0=pe_sb,
                                            in1=zm[:, 1, :], op=OP.add)
                    return None

            for (j0, bj) in blocks:
                jlist = list(range(j0, j0 + bj))
                xg = {}
                gi = 0
                jgp = j0
                while jgp < j0 + bj:
                    gw = min(JG, j0 + bj - jgp)
                    xt = xload.tile([H, 5, gw * P], F16, tag=f"x_{gi}_{gw}")
                    nc.sync.dma_start(
                        out=xt, in_=xt5[:, :, jgp * P:(jgp + gw) * P])
                    for p in range(5):
                        for jj in range(gw):
                            xg[(jgp + jj, p)] = xt[:, p, jj * P:(jj + 1) * P]
                    gi += 1
                    jgp += gw

                st = sload.tile([P, bj * P], F16, tag=f"s16blk_{bj}")
                nc.sync.dma_start(out=st, in_=s16d[:, j0 * P:(j0 + bj) * P])
                s16_by_j = {}
                for j in jlist:
                    jj = j - j0
                    s16_by_j[j] = st[:, jj * P:(jj + 1) * P]

                lhs_by_j = {j: [xg[(j, p)] for p in range(5)] for j in jlist}

                for s in range(3):
                    S1 = SKEW if isinstance(SKEW, int) else SKEW[0]
                    S2 = (S1 + 1) if isinstance(SKEW, int) else SKEW[1]
                    stA, stB = {}, {}
                    aggpairs, rquake = {}, {}
                    npair_of = {}

                    def do_b1(i_):
                        jj_ = jlist[i_]
                        stB[jj_] = emit_B1(jj_, s, stA.pop(jj_), aggpairs,
                                           rquake, i_ // 2, i_ % 2,
                                           npair_of[i_ // 2])

                    def do_b2(i_):
                        jj_ = jlist[i_]
                        nl = emit_B2(jj_, s, stB.pop(jj_), s16_by_j[jj_])
                        if nl is not None:
                            lhs_by_j[jj_] = nl

                    for idx, j in enumerate(jlist):
                        npair_of[idx // 2] = min(2, len(jlist) - (idx // 2) * 2)
                        stA[j] = emit_A(j, s, lhs_by_j[j], aggpairs,
                                        idx // 2, idx % 2)
                        if idx >= S1:
                            do_b1(idx - S1)
                        if idx >= S2:
                            do_b2(idx - S2)
                    n = len(jlist)
                    for idx in range(n, n + S2):
                        if 0 <= idx - S1 < n and jlist[idx - S1] in stA:
                            do_b1(idx - S1)
                        if 0 <= idx - S2 < n and jlist[idx - S2] in stB:
                            do_b2(idx - S2)

            # --- final projection ---
            ppf = singles.tile([P, T, H], F32, tag="ppf")
            nc.vector.tensor_copy(out=ppf, in_=pp)
            ppf_flat = bass.AP(tensor=ppf.tensor, offset=ppf.offset,
                               ap=[ppf.ap[0], [1, T * H]])
            ppm = singles.tile([P, T * H], F32, tag="ppm")
            nc.vector.tensor_tensor(out=ppm, in0=ppf_flat,
                                    in1=wp4t, op=OP.mult)
            pem = singles.tile([P, H], F32, tag="pem")
            nc.vector.tensor_tensor(out=pem, in0=pe_sb, in1=webt,
                                    op=OP.mult)
            projp = singles.tile([P, T], F32, tag="projp")
            nc.vector.reduce_sum(out=projp,
                                 in_=ppm.rearrange("p (a b) -> p a b", a=T),
                                 axis=mybir.AxisListType.X)
            proje = singles.tile([P, 1], F32, tag="proje")
            nc.vector.reduce_sum(out=proje, in_=pem, axis=mybir.AxisListType.X)
            rest = singles.tile([P, T], F32, tag="rest")
            nc.vector.tensor_scalar(out=rest, in0=projp, scalar1=proje,
                                    scalar2=kvt, op0=OP.add, op1=OP.add)
            nc.sync.dma_start(out=res, in_=rest)

    nc.compile()
    return nc


class _Runner:
    """Holds the jitted PJRT callable for repeated execution."""

    def __init__(self, nc, n_cores):
        import jax
        from jax.sharding import Mesh, PartitionSpec
        from jax.experimental.shard_map import shard_map
        from concourse import mybir
        from concourse.bass2jax import (_bass_exec_p, install_neuronx_cc_hook,
                                        partition_id_tensor)
        install_neuronx_cc_hook()
        self.jax = jax
        self.n_cores = n_cores
        partition_name = nc.partition_id_tensor.name if nc.partition_id_tensor else None
        dbg_name = nc.dbg_addr.name if nc.dbg_addr else None
        in_names, out_names, out_avals, zero_outs = [], [], [], []
        for alloc in nc.m.functions[0].allocations:
            if not isinstance(alloc, mybir.MemoryLocationSet):
                continue
            name = alloc.memorylocations[0].name
            if alloc.kind == "ExternalInput":
                if name not in (partition_name, dbg_name):
                    in_names.append(name)
            elif alloc.kind == "ExternalOutput":
                shape = tuple(alloc.tensor_shape)
                dtype = mybir.dt.np(alloc.dtype)
                out_names.append(name)
                out_avals.append(jax.core.ShapedArray(shape, dtype))
                zero_outs.append(np.zeros(shape, dtype))
        self.in_names, self.out_names = in_names, out_names
        self.out_avals, self.zero_outs = out_avals, zero_outs
        all_in = list(in_names) + list(out_names)
        if dbg_name is not None:
            all_in.append(dbg_name)
        if partition_name is not None:
            all_in.append(partition_name)

        def _body(*args):
            operands = list(args)
            if dbg_name is not None:
                operands.append(jax.numpy.zeros((1, 2), jax.numpy.uint32))
            if partition_name is not None:
                operands.append(partition_id_tensor())
            return tuple(_bass_exec_p.bind(
                *operands, out_avals=tuple(out_avals), in_names=tuple(all_in),
                out_names=tuple(out_names), lowering_input_output_aliases=(),
                sim_require_finite=True, sim_require_nnan=True, nc=nc))

        devices = jax.devices()[:n_cores]
        self.mesh = Mesh(np.asarray(devices), ("core",))
        n_io = len(in_names) + len(out_names)
        self.fn = jax.jit(
            shard_map(_body, mesh=self.mesh,
                      in_specs=(PartitionSpec("core"),) * n_io,
                      out_specs=(PartitionSpec("core"),) * len(out_names),
                      check_rep=False),
            keep_unused=True)

    def prepare(self, in_maps):
        import jax
        from jax.sharding import PartitionSpec
        n = self.n_cores
        sharding = jax.sharding.NamedSharding(self.mesh, PartitionSpec("core"))
        dev_in = [jax.device_put(
            np.concatenate([np.asarray(in_maps[c][name]) for c in range(n)], axis=0),
            sharding) for name in self.in_names]
        dev_zero = [jax.device_put(
            np.zeros((n * z.shape[0], *z.shape[1:]), z.dtype), sharding)
            for z in self.zero_outs]
        return dev_in, dev_zero

    def run(self, handle):
        dev_in, dev_zero = handle
        outs = self.fn(*dev_in, *dev_zero)
        self.jax.block_until_ready(outs)
        return outs

    def results(self, outs):
        n = self.n_cores
        return [{name: np.asarray(outs[i]).reshape(n, *self.out_avals[i].shape)[c]
                 for i, name in enumerate(self.out_names)} for c in range(n)]


def _prep_inputs(x_proc, x_enc, batch, pW, pb, pg, pbt, eW, eb, eg, ebt,
                 wp, bp, we, be):
    """Host-side sharding + precomputation. Returns (in_maps, meta)."""
    x_proc = np.asarray(x_proc, dtype=np.float32)
    x_enc = np.asarray(x_enc, dtype=np.float32)
    batch = np.asarray(batch).astype(np.int64)
    pW = np.asarray(pW, dtype=np.float32)
    eW = np.asarray(eW, dtype=np.float32)
    pb = np.asarray(pb, dtype=np.float32)
    eb = np.asarray(eb, dtype=np.float32)
    wp = np.asarray(wp, dtype=np.float32).reshape(H)
    we = np.asarray(we, dtype=np.float32).reshape(H)
    bp = float(np.asarray(bp).reshape(-1)[0])
    be = float(np.asarray(be).reshape(-1)[0])

    assert np.allclose(np.asarray(pg), 1) and np.allclose(np.asarray(eg), 1), \
        "kernel assumes LN gain == 1"
    assert np.allclose(np.asarray(pbt), 0) and np.allclose(np.asarray(ebt), 0), \
        "kernel assumes LN shift == 0"

    splits = np.searchsorted(batch, np.arange(NCORES + 1) * SEG_PER_CORE)
    rows = splits[1:] - splits[:-1]
    nloc_raw = int(rows.max())
    nsub = max(1, -(-nloc_raw // P))
    nloc = nsub * P

    def center16(W):
        # fold LN's mean subtraction into the weights
        return (W - W.mean(axis=-1, keepdims=True)).astype(np.float16)

    pw16 = np.stack([center16(pW[i]) for i in range(3)])
    ew16 = np.stack([center16(eW[i]) for i in range(3)])

    def beff(b, W16):
        e = np.stack([b[0],
                      b[1] - LN2 * W16[1].astype(np.float32).sum(0),
                      b[2] - LN2 * W16[2].astype(np.float32).sum(0)])
        return e - e.mean(axis=-1, keepdims=True)

    pb_eff = beff(pb, pw16)
    eb_eff = beff(eb, ew16)
    use_bias0p = bool(np.abs(pb_eff[0]).max() > 1e-7)
    use_bias0e = bool(np.abs(eb_eff[0]).max() > 1e-7)

    ident = np.eye(H, dtype=np.float16)
    wp4b = np.tile(wp[None, :], (P, T)).astype(np.float32)        # [P, T*H]
    web = np.tile(we[None, :], (P, 1)).astype(np.float32)         # [P, H]

    l16, n16, l32, n32 = _blob_layout(nsub)

    def pack(blob, layout, name, arr):
        off, shape = layout[name]
        assert tuple(arr.shape) == tuple(shape), (name, arr.shape, shape)
        blob[off:off + arr.size] = arr.ravel()

    in_maps = []
    for c in range(NCORES):
        lo, hi = int(splits[c]), int(splits[c + 1])
        n_c = hi - lo
        xt5 = np.zeros((H, 5, nloc), np.float16)
        xt5[:, 0:T, :n_c] = x_proc[:, lo:hi, :].transpose(2, 0, 1).astype(np.float16)
        xt5[:, T, :n_c] = x_enc[lo:hi, :].T.astype(np.float16)
        bl = np.full(nloc, -1.0, np.float32)
        bl[:n_c] = (batch[lo:hi] - c * SEG_PER_CORE).astype(np.float32)
        blj = bl.reshape(nsub, P)
        oh = (blj[:, :, None] == np.arange(P, dtype=np.float32)[None, None, :])
        s16d = np.ascontiguousarray(
            oh.transpose(1, 0, 2).reshape(P, nsub * P)).astype(np.float16)
        cnt = np.zeros(SEG_PER_CORE, np.float64)
        segs, counts = np.unique(batch[lo:hi], return_counts=True)
        cnt[(segs - c * SEG_PER_CORE).astype(int)] = counts
        kv = (bp + be - LN2 * cnt * (wp.sum() + we.sum())).astype(np.float32)
        blob16 = np.zeros(n16, np.float16)
        pack(blob16, l16, "xt5", xt5)
        pack(blob16, l16, "s16d", s16d)
        pack(blob16, l16, "pw16", pw16)
        pack(blob16, l16, "ew16", ew16)
        pack(blob16, l16, "pb16", pb_eff.astype(np.float16))
        pack(blob16, l16, "eb16", eb_eff.astype(np.float16))
        pack(blob16, l16, "ident", ident)
        blob32 = np.zeros(n32, np.float32)
        pack(blob32, l32, "wp4b", wp4b)
        pack(blob32, l32, "web", web)
        pack(blob32, l32, "kvec", kv.reshape(P, 1).astype(np.float32))
        in_maps.append({"b16": blob16, "b32": blob32})
    meta = (nsub, use_bias0p, use_bias0e)
    return in_maps, meta


def get_runner(meta):
    key = meta
    if key not in _cache:
        nc = _build(*meta)
        _cache[key] = _Runner(nc, NCORES)
    return _cache[key]


def kernel(**inputs) -> np.ndarray:
    in_maps, meta = _prep_inputs(**inputs)
    runner = get_runner(meta)
    handle = runner.prepare(in_maps)
    outs = runner.run(handle)
    per_core = runner.results(outs)
    out = np.zeros((T, G), np.float32)
    for c in range(NCORES):
        out[:, c * SEG_PER_CORE:(c + 1) * SEG_PER_CORE] = per_core[c]["res"].T
    return out


# revision 39
# speedup vs baseline: 1.2905x; 1.2905x over previous
"""Trainium2 Bass kernel for nn_Decoder (mlp3 + segment_sum decoder), 8 cores.

Strategy (data-parallel over nodes N, segment-aligned shard boundaries):
  - Host: shard rows so core c owns whole segments [128c, 128(c+1)); transpose
    x to [H, rows] fp16 so stage-1 matmul lhsT streams straight from HBM;
    precompute folded biases (the ssp "-ln2" shift folded into the next
    stage's bias / the final projection's per-segment count correction).
    ALL inputs are packed into two DRAM blobs (fp16 + f32): each extra
    PJRT/axon buffer costs ~50us of per-run dispatch overhead.
  - Device (per core, SPMD), per 128-row subtile j, per stage s:
      matmul (fp16, weights moving) -> z in PSUM f32 (+rank-1 bias matmul);
      variance via bn_stats/bn_aggr (the only single-PSUM-read reduce);
      rstd = exp(-0.5*ln(var+eps)) on ACT (2 tiny ops, one act-table set);
      normalize+exp fused in ONE ACT pass: Exp(z, scale=rstd) -> fp16 SBUF;
      stages 0/1: PE transpose (fp16 PSUM out, halves the transpose bank and
      enables a single batched Ln) then ONE batched ACT Ln(1+x) [P,5H]
      -> fp16 SBUF as the next stage's lhsT (the Ln doubles as the
      PSUM->SBUF mover, so there is no separate copy pass);
      stage 2: batched Ln then one-hot segment matmul accumulating pooled
      proc sums in PSUM; enc partials land in a scratch bank and are
      accumulated into SBUF by DVE.
    Three-phase software pipeline (A: matmul+stats, B1: rstd+exp,
    B2: transpose+ln+pool) with independent skews keeps PE/DVE/ACT
    overlapped; PSUM budget: 4 z-banks + 1 transpose + 2 misc + 1 acc.
  - Final tiny projection (pooled @ wp/we + consts) on device; host gathers
    the per-core [128 segs, T] outputs into [T, G].
"""
import sys
sys.path.insert(0, "/opt/trn_rl_repo")
import numpy as np

T, N, H, G = 4, 100000, 128, 1024
NCORES = 8
SEG_PER_CORE = G // NCORES        # 128
LN2 = float(np.log(2.0))
EPS = 1e-5
P = 128

# tunable build config
CFG = {
    "BJ": 16,       # j's interleaved per pipeline block
    "JG": 8,        # subtiles loaded per DMA group (one dma_start each)
    "skew": (2, 5),  # software pipeline depth (B1, B2)
    # per-(j,s) normalize route: "U" ACT-scaled Exp x5 (cheap on DVE);
    # "D" DVE normalize + one batched ACT Exp (cheap on ACT).
    "route": lambda j, s: "U",
    "zp_bufs": 4,   # PSUM z4 double-buffer depth
    "et_bufs": 1,   # PSUM transpose-out buffer depth
    "rstd": "act",  # "act" = exp(-.5 ln v) on ACT (pair-batched); "quake" = DVE rsqrt
}

_cache = {}


def _blob_layout(nsub):
    """Offsets (in elements) of each logical tensor inside the two input
    blobs. Packing everything into 2 DRAM buffers cuts per-run dispatch
    overhead (~50us per buffer per run on the PJRT/axon path)."""
    nloc = nsub * P
    l16, o = {}, 0
    for name, shape in (("xt5", (H, 5, nloc)), ("s16d", (P, nloc)),
                        ("pw16", (3, H, H)), ("ew16", (3, H, H)),
                        ("pb16", (3, H)), ("eb16", (3, H)),
                        ("ident", (H, H))):
        l16[name] = (o, shape)
        o += int(np.prod(shape))
    n16 = o
    l32, o = {}, 0
    for name, shape in (("wp4b", (P, T * H)), ("web", (P, H)),
                        ("kvec", (P, 1))):
        l32[name] = (o, shape)
        o += int(np.prod(shape))
    return l16, n16, l32, o


_tables_patched = False


def _patch_act_tables():
    """Make the table chooser put Exp and Ln in one set (one load total)."""
    global _tables_patched
    if _tables_patched:
        return
    import concourse.bacc as bacc
    from concourse import mybir, hw_specs
    _orig = hw_specs.get_activation_tables

    def patched(arch):
        t = _orig(arch)
        AF = mybir.ActivationFunctionType
        for name, funcs in t.items():
            if name != "natural_log_exp_and_others":
                t[name] = {f for f in funcs if f not in (AF.Exp, AF.Ln)}
        return t

    bacc.get_activation_tables = patched
    _tables_patched = True


def _build(nsub, use_bias0p, use_bias0e, cfg=None):
    import concourse.bass as bass
    import concourse.bacc as bacc
    import concourse.tile as tile
    from concourse import mybir
    _patch_act_tables()
    cfg = dict(CFG, **(cfg or {}))
    route = cfg["route"]
    F16, F32 = mybir.dt.float16, mybir.dt.float32
    I32 = mybir.dt.int32
    AF = mybir.ActivationFunctionType
    OP = mybir.AluOpType
    MAGIC = 0x5f375a86

    BJ = cfg["BJ"]
    JG = cfg["JG"]
    SKEW = cfg["skew"]
    assert BJ % JG == 0
    blocks = []
    jb0 = 0
    while jb0 < nsub:
        blocks.append((jb0, min(BJ, nsub - jb0)))
        jb0 += BJ
    nc = bacc.Bacc("TRN2", target_bir_lowering=False, debug=False,
                   enable_asserts=True, num_devices=NCORES)

    l16, n16, l32, n32 = _blob_layout(nsub)
    blob16 = nc.dram_tensor("b16", [n16], F16, kind="ExternalInput").ap()
    blob32 = nc.dram_tensor("b32", [n32], F32, kind="ExternalInput").ap()
    res = nc.dram_tensor("res", [P, T], F32, kind="ExternalOutput").ap()

    def _sub(blob, layout, name):
        off, shape = layout[name]
        ap = [[int(np.prod(shape[i + 1:])), shape[i]] for i in range(len(shape))]
        return bass.AP(tensor=blob.tensor, offset=blob.offset + off, ap=ap)

    xt5 = _sub(blob16, l16, "xt5")
    s16d = _sub(blob16, l16, "s16d")
    pw16 = _sub(blob16, l16, "pw16")
    ew16 = _sub(blob16, l16, "ew16")
    pb16 = _sub(blob16, l16, "pb16")
    eb16 = _sub(blob16, l16, "eb16")
    ident = _sub(blob16, l16, "ident")
    wp4b = _sub(blob32, l32, "wp4b")
    web = _sub(blob32, l32, "web")
    kvec = _sub(blob32, l32, "kvec")

    with tile.TileContext(nc) as tc:
        import contextlib
        with contextlib.ExitStack() as ctx:
            singles = ctx.enter_context(tc.tile_pool(name="singles", bufs=1))
            xload = ctx.enter_context(tc.tile_pool(name="xload", bufs=4))
            sload = ctx.enter_context(tc.tile_pool(name="sload", bufs=3))
            work = ctx.enter_context(tc.tile_pool(name="work", bufs=6))
            scrp = ctx.enter_context(tc.tile_pool(name="scr", bufs=2))
            blockp = ctx.enter_context(tc.tile_pool(name="blockp", bufs=12))
            stat = ctx.enter_context(tc.tile_pool(name="stat", bufs=16))
            zpool = ctx.enter_context(tc.tile_pool(name="zp", bufs=cfg["zp_bufs"], space="PSUM"))
            etpool = ctx.enter_context(tc.tile_pool(name="etp", bufs=cfg["et_bufs"], space="PSUM"))
            mis = ctx.enter_context(tc.tile_pool(name="mis", bufs=2, space="PSUM"))
            acc = ctx.enter_context(tc.tile_pool(name="acc", bufs=1, space="PSUM"))

            # --- one-time constants ---
            w16 = []
            for fam, src in (("p", pw16), ("e", ew16)):
                fam_tiles = []
                for si in range(3):
                    wt = singles.tile([H, H], F16, tag=f"w{fam}{si}")
                    nc.sync.dma_start(out=wt, in_=src[si])
                    fam_tiles.append(wt)
                w16.append(fam_tiles)
            b16 = []
            for fam, src in (("p", pb16), ("e", eb16)):
                fam_tiles = []
                for si in range(3):
                    bt = singles.tile([1, H], F16, tag=f"b{fam}{si}")
                    nc.sync.dma_start(out=bt, in_=src[si:si + 1, :])
                    fam_tiles.append(bt)
                b16.append(fam_tiles)
            i16 = singles.tile([H, H], F16, tag="ident")
            nc.sync.dma_start(out=i16, in_=ident)
            ones16 = singles.tile([1, H], F16, tag="ones")
            nc.vector.memset(ones16, 1.0)
            eps_t = singles.tile([P, 1], F32, tag="eps")
            nc.vector.memset(eps_t, EPS)
            wp4t = singles.tile([P, T * H], F32, tag="wp4")
            nc.sync.dma_start(out=wp4t, in_=wp4b)
            webt = singles.tile([P, H], F32, tag="web")
            nc.sync.dma_start(out=webt, in_=web)
            kvt = singles.tile([P, 1], F32, tag="kv")
            nc.sync.dma_start(out=kvt, in_=kvec)

            # --- pooled accumulators: proc in PSUM, enc partials added into SBUF
            pp = acc.tile([P, T, H], F32, tag="pp")
            pp_proc = bass.AP(tensor=pp.tensor, offset=pp.offset,
                              ap=[pp.ap[0], [1, T * H]])
            pe_sb = singles.tile([P, H], F32, tag="pe_sb")
            nc.vector.memset(pe_sb, 0.0)

            use_bias = [[use_bias0p, True, True], [use_bias0e, True, True]]

            rstate = {}   # j -> (zs, v5)

            def emit_A(j, s, lhs, aggpairs, pi, sl):
                z4 = zpool.tile([P, T * H], F32, tag="z4")
                zm = mis.tile([P, 2, H], F32, tag="zm")
                zs = [z4[:, p * H:(p + 1) * H] for p in range(T)] + [zm[:, 0, :]]
                for p in range(5):
                    fam = 0 if p < T else 1
                    has_b = use_bias[fam][s]
                    nc.tensor.matmul(zs[p], lhsT=lhs[p], rhs=w16[fam][s],
                                     start=True, stop=not has_b)
                    if has_b:
                        nc.tensor.matmul(zs[p], lhsT=ones16, rhs=b16[fam][s],
                                         start=False, stop=True)
                stc = stat.tile([P, 5, 6], F32, tag="stc6")
                if sl == 0:
                    aggp = stat.tile([P, 2, 5, 2], F32, tag="aggp")
                    aggpairs[pi] = aggp
                agg = aggpairs[pi]
                for p in range(5):
                    nc.vector.bn_stats(out=stc[:, p, :], in_=zs[p])
                    nc.vector.bn_aggr(out=agg[:, sl, p, :], in_=stc[:, p, :])
                r5A = None
                if cfg["rstd"] == "quakeA":
                    # quake rsqrt emitted right here: stays DVE-local, so the
                    # ACT Exp in B1 never waits on a cross-engine rstd chain
                    agg1 = bass.AP(tensor=agg.tensor, offset=agg.offset + sl * 10,
                                   ap=[agg.ap[0], [2, 5], [1, 2]])
                    r5A = emit_rstd_quake_ap(agg1)
                return zs, zm, r5A

            def emit_rstd_quake_ap(agg1):
                """DVE bit-hack rsqrt of (var + eps); agg1 = [P,5,2] AP."""
                nv = 5
                var = bass.AP(tensor=agg1.tensor, offset=agg1.offset + 1,
                              ap=[agg1.ap[0], [2, nv]])
                return _quake_chain(var, nv)

            def emit_rstd_quake(agg, npair):
                """DVE bit-hack rsqrt of (var + eps) for `npair` j's at once."""
                nv = 5 * npair
                var = bass.AP(tensor=agg.tensor, offset=agg.offset + 1,
                              ap=[agg.ap[0], [2, nv]])
                return _quake_chain(var, nv)

            def _quake_chain(var, nv):
                vpe = stat.tile([P, nv], F32, tag="vpe")
                nc.vector.tensor_scalar(out=vpe, in0=var, scalar1=EPS,
                                        scalar2=None, op0=OP.add)
                t1 = stat.tile([P, nv], I32, tag="qt1")
                nc.vector.tensor_scalar(out=t1, in0=vpe.bitcast(I32),
                                        scalar1=1, scalar2=None,
                                        op0=OP.logical_shift_right)
                t1n = stat.tile([P, nv], I32, tag="qt1n")
                nc.vector.tensor_scalar(out=t1n, in0=t1, scalar1=0,
                                        scalar2=None, op0=OP.bitwise_not)
                t2 = stat.tile([P, nv], I32, tag="qt2")
                nc.vector.tensor_scalar(out=t2, in0=t1n, scalar1=MAGIC + 1,
                                        scalar2=None, op0=OP.add)
                y = t2.bitcast(F32)
                for it in range(2):
                    a = stat.tile([P, nv], F32, tag=f"qa{it}")
                    nc.vector.tensor_tensor(out=a, in0=y, in1=y, op=OP.mult)
                    b = stat.tile([P, nv], F32, tag=f"qb{it}")
                    nc.vector.scalar_tensor_tensor(out=b, in0=a, scalar=-0.5,
                                                   in1=vpe, op0=OP.mult,
                                                   op1=OP.mult)
                    y2 = stat.tile([P, nv], F32, tag=f"qy{it}")
                    nc.vector.scalar_tensor_tensor(out=y2, in0=b, scalar=1.5,
                                                   in1=y, op0=OP.add,
                                                   op1=OP.mult)
                    y = y2
                return y

            def emit_B1(j, s, state, aggpairs, rquake, pi, sl, npair):
                zs, zm, r5A = state
                agg = aggpairs[pi]
                if cfg["rstd"] == "quakeA":
                    r5 = r5A
                elif cfg["rstd"] == "quake":
                    if pi not in rquake:
                        rquake[pi] = emit_rstd_quake(agg, npair)
                    r5 = rquake[pi][:, sl * 5: sl * 5 + 5]
                elif cfg["rstd"] == "act2":
                    # rstd = exp(-0.5 * ln(var + eps)) on ACT, batched per pair
                    if pi not in rquake:
                        nv = 5 * npair
                        var = bass.AP(tensor=agg.tensor, offset=agg.offset + 1,
                                      ap=[agg.ap[0], [2, nv]])
                        l10 = stat.tile([P, nv], F32, tag="l10")
                        nc.scalar.activation(out=l10, in_=var, func=AF.Ln,
                                             bias=eps_t, scale=1.0)
                        r10 = stat.tile([P, nv], F32, tag="r10")
                        nc.scalar.activation(out=r10, in_=l10, func=AF.Exp,
                                             bias=0.0, scale=-0.5)
                        rquake[pi] = r10
                    r5 = rquake[pi][:, sl * 5: sl * 5 + 5]
                else:
                    # per-js rstd on ACT (lowest-latency chain)
                    var5 = bass.AP(tensor=agg.tensor,
                                   offset=agg.offset + sl * 10 + 1,
                                   ap=[agg.ap[0], [2, 5]])
                    l5 = stat.tile([P, 5], F32, tag="l5")
                    nc.scalar.activation(out=l5, in_=var5, func=AF.Ln,
                                         bias=eps_t, scale=1.0)
                    r5 = stat.tile([P, 5], F32, tag="r5")
                    nc.scalar.activation(out=r5, in_=l5, func=AF.Exp,
                                         bias=0.0, scale=-0.5)
                mode = route(j, s)
                e5 = work.tile([P, 5, H], F16, tag="e5")
                if mode == "U":
                    for p in range(5):
                        nc.scalar.activation(out=e5[:, p, :], in_=zs[p],
                                             func=AF.Exp, bias=0.0,
                                             scale=r5[:, p:p + 1])
                else:
                    zn5 = work.tile([P, 5, H], F16, tag="zn5")
                    for p in range(5):
                        nc.vector.tensor_scalar(
                            out=zn5[:, p, :], in0=zs[p],
                            scalar1=r5[:, p:p + 1], scalar2=None,
                            op0=OP.mult)
                    nc.scalar.activation(out=e5, in_=zn5, func=AF.Exp,
                                         bias=0.0, scale=1.0)
                return e5, zm

            def emit_B2(j, s, state, s16s):
                e5, zm = state
                if s < 2:
                    et5 = etpool.tile([P, 5, H], F16, tag="et5")
                    for p in range(5):
                        nc.tensor.transpose(et5[:, p, :], e5[:, p, :], i16)
                    at5 = blockp.tile([P, 5, H], F16, tag="at5")
                    nc.scalar.activation(out=at5, in_=et5, func=AF.Ln,
                                         bias=1.0, scale=1.0)
                    return [at5[:, p, :] for p in range(5)]
                else:
                    a5 = work.tile([P, 5, H], F16, tag="a5")
                    nc.scalar.activation(out=a5, in_=e5, func=AF.Ln,
                                         bias=1.0, scale=1.0)
                    a4_flat = bass.AP(tensor=a5.tensor, offset=a5.offset,
                                      ap=[a5.ap[0], [1, T * H]])
                    nc.tensor.matmul(pp_proc, lhsT=s16s, rhs=a4_flat,
                                     start=(j == 0), stop=(j == nsub - 1))
                    # enc pooled partial -> spare zm slot -> SBUF accumulator
                    nc.tensor.matmul(zm[:, 1, :], lhsT=s16s, rhs=a5[:, T, :],
                                     start=True, stop=True)
                    nc.vector.tensor_tensor(out=pe_sb, in0=pe_sb,
                                            in1=zm[:, 1, :], op=OP.add)
                    return None

            for (j0, bj) in blocks:
                jlist = list(range(j0, j0 + bj))
                xg = {}
                gi = 0
                jgp = j0
                while jgp < j0 + bj:
                    gw = min(JG, j0 + bj - jgp)
                    xt = xload.tile([H, 5, gw * P], F16, tag=f"x_{gi}_{gw}")
                    nc.sync.dma_start(
                        out=xt, in_=xt5[:, :, jgp * P:(jgp + gw) * P])
                    for p in range(5):
                        for jj in range(gw):
                            xg[(jgp + jj, p)] = xt[:, p, jj * P:(jj + 1) * P]
                    gi += 1
                    jgp += gw

                st = sload.tile([P, bj * P], F16, tag=f"s16blk_{bj}")
                nc.sync.dma_start(out=st, in_=s16d[:, j0 * P:(j0 + bj) * P])
                s16_by_j = {}
                for j in jlist:
                    jj = j - j0
                    s16_by_j[j] = st[:, jj * P:(jj + 1) * P]

                lhs_by_j = {j: [xg[(j, p)] for p in range(5)] for j in jlist}

                for s in range(3):
                    S1 = SKEW if isinstance(SKEW, int) else SKEW[0]
                    S2 = (S1 + 1) if isinstance(SKEW, int) else SKEW[1]
                    stA, stB = {}, {}
                    aggpairs, rquake = {}, {}
                    npair_of = {}

                    def do_b1(i_):
                        jj_ = jlist[i_]
                        stB[jj_] = emit_B1(jj_, s, stA.pop(jj_), aggpairs,
                                           rquake, i_ // 2, i_ % 2,
                                           npair_of[i_ // 2])

                    def do_b2(i_):
                        jj_ = jlist[i_]
                        nl = emit_B2(jj_, s, stB.pop(jj_), s16_by_j[jj_])
                        if nl is not None:
                            lhs_by_j[jj_] = nl

                    for idx, j in enumerate(jlist):
                        npair_of[idx // 2] = min(2, len(jlist) - (idx // 2) * 2)
                        stA[j] = emit_A(j, s, lhs_by_j[j], aggpairs,
                                        idx // 2, idx % 2)
                        if idx >= S1:
                            do_b1(idx - S1)
                        if idx >= S2:
                            do_b2(idx - S2)
                    n = len(jlist)
                    for idx in range(n, n + S2):
                        if 0 <= idx - S1 < n and jlist[idx - S1] in stA:
                            do_b1(idx - S1)
                        if 0 <= idx - S2 < n and jlist[idx - S2] in stB:
                            do_b2(idx - S2)

            # --- final projection ---
            ppf = singles.tile([P, T, H], F32, tag="ppf")
            nc.vector.tensor_copy(out=ppf, in_=pp)
            ppf_flat = bass.AP(tensor=ppf.tensor, offset=ppf.offset,
                               ap=[ppf.ap[0], [1, T * H]])
            ppm = singles.tile([P, T * H], F32, tag="ppm")
            nc.vector.tensor_tensor(out=ppm, in0=ppf_flat,
                                    in1=wp4t, op=OP.mult)
            pem = singles.tile([P, H], F32, tag="pem")
            nc.vector.tensor_tensor(out=pem, in0=pe_sb, in1=webt,
                                    op=OP.mult)
            projp = singles.tile([P, T], F32, tag="projp")
            nc.vector.reduce_sum(out=projp,
                                 in_=ppm.rearrange("p (a b) -> p a b", a=T),
                                 axis=mybir.AxisListType.X)
            proje = singles.tile([P, 1], F32, tag="proje")
            nc.vector.reduce_sum(out=proje, in_=pem, axis=mybir.AxisListType.X)
            rest = singles.tile([P, T], F32, tag="rest")
            nc.vector.tensor_scalar(out=rest, in0=projp, scalar1=proje,
                                    scalar2=kvt, op0=OP.add, op1=OP.add)
            nc.sync.dma_start(out=res, in_=rest)

    nc.compile()
    return nc


class _Runner:
    """Holds the jitted PJRT callable for repeated execution."""

    def __init__(self, nc, n_cores):
        import jax
        from jax.sharding import Mesh, PartitionSpec
        from jax.experimental.shard_map import shard_map
        from concourse import mybir
        from concourse.bass2jax import (_bass_exec_p, install_neuronx_cc_hook,
                                        partition_id_tensor)
        install_neuronx_cc_hook()
        self.jax = jax
        self.n_cores = n_cores
        partition_name = nc.partition_id_tensor.name if nc.partition_id_tensor else None
        dbg_name = nc.dbg_addr.name if nc.dbg_addr else None
        in_names, out_names, out_avals, zero_outs = [], [], [], []
        for alloc in nc.m.functions[0].allocations:
            if not isinstance(alloc, mybir.MemoryLocationSet):
                continue
            name = alloc.memorylocations[0].name
            if alloc.kind == "ExternalInput":
                if name not in (partition_name, dbg_name):
                    in_names.append(name)
            elif alloc.kind == "ExternalOutput":
                shape = tuple(alloc.tensor_shape)
                dtype = mybir.dt.np(alloc.dtype)
                out_names.append(name)
                out_avals.append(jax.core.ShapedArray(shape, dtype))
                zero_outs.append(np.zeros(shape, dtype))
        self.in_names, self.out_names = in_names, out_names
        self.out_avals, self.zero_outs = out_avals, zero_outs
        all_in = list(in_names) + list(out_names)
        if dbg_name is not None:
            all_in.append(dbg_name)
        if partition_name is not None:
            all_in.append(partition_name)

        def _body(*args):
            operands = list(args)
            if dbg_name is not None:
                operands.append(jax.numpy.zeros((1, 2), jax.numpy.uint32))
            if partition_name is not None:
                operands.append(partition_id_tensor())
            return tuple(_bass_exec_p.bind(
                *operands, out_avals=tuple(out_avals), in_names=tuple(all_in),
                out_names=tuple(out_names), lowering_input_output_aliases=(),
                sim_require_finite=True, sim_require_nnan=True, nc=nc))

        devices = jax.devices()[:n_cores]
        self.mesh = Mesh(np.asarray(devices), ("core",))
        n_io = len(in_names) + len(out_names)
        self.fn = jax.jit(
            shard_map(_body, mesh=self.mesh,
                      in_specs=(PartitionSpec("core"),) * n_io,
                      out_specs=(PartitionSpec("core"),) * len(out_names),
                      check_rep=False),
            keep_unused=True)

    def prepare(self, in_maps):
        import jax
        from jax.sharding import PartitionSpec
        n = self.n_cores
        sharding = jax.sharding.NamedSharding(self.mesh, PartitionSpec("core"))
        dev_in = [jax.device_put(
            np.concatenate([np.asarray(in_maps[c][name]) for c in range(n)], axis=0),
            sharding) for name in self.in_names]
        dev_zero = [jax.device_put(
            np.zeros((n * z.shape[0], *z.shape[1:]), z.dtype), sharding)
            for z in self.zero_outs]
        return dev_in, dev_zero

    def run(self, handle):
        dev_in, dev_zero = handle
        outs = self.fn(*dev_in, *dev_zero)
        self.jax.block_until_ready(outs)
        return outs

    def results(self, outs):
        n = self.n_cores
        return [{name: np.asarray(outs[i]).reshape(n, *self.out_avals[i].shape)[c]
                 for i, name in enumerate(self.out_names)} for c in range(n)]


def _prep_inputs(x_proc, x_enc, batch, pW, pb, pg, pbt, eW, eb, eg, ebt,
                 wp, bp, we, be):
    """Host-side sharding + precomputation. Returns (in_maps, meta)."""
    x_proc = np.asarray(x_proc, dtype=np.float32)
    x_enc = np.asarray(x_enc, dtype=np.float32)
    batch = np.asarray(batch).astype(np.int64)
    pW = np.asarray(pW, dtype=np.float32)
    eW = np.asarray(eW, dtype=np.float32)
    pb = np.asarray(pb, dtype=np.float32)
    eb = np.asarray(eb, dtype=np.float32)
    wp = np.asarray(wp, dtype=np.float32).reshape(H)
    we = np.asarray(we, dtype=np.float32).reshape(H)
    bp = float(np.asarray(bp).reshape(-1)[0])
    be = float(np.asarray(be).reshape(-1)[0])

    assert np.allclose(np.asarray(pg), 1) and np.allclose(np.asarray(eg), 1), \
        "kernel assumes LN gain == 1"
    assert np.allclose(np.asarray(pbt), 0) and np.allclose(np.asarray(ebt), 0), \
        "kernel assumes LN shift == 0"

    splits = np.searchsorted(batch, np.arange(NCORES + 1) * SEG_PER_CORE)
    rows = splits[1:] - splits[:-1]
    nloc_raw = int(rows.max())
    nsub = max(1, -(-nloc_raw // P))
    nloc = nsub * P

    def center16(W):
        # fold LN's mean subtraction into the weights
        return (W - W.mean(axis=-1, keepdims=True)).astype(np.float16)

    pw16 = np.stack([center16(pW[i]) for i in range(3)])
    ew16 = np.stack([center16(eW[i]) for i in range(3)])

    def beff(b, W16):
        e = np.stack([b[0],
                      b[1] - LN2 * W16[1].astype(np.float32).sum(0),
                      b[2] - LN2 * W16[2].astype(np.float32).sum(0)])
        return e - e.mean(axis=-1, keepdims=True)

    pb_eff = beff(pb, pw16)
    eb_eff = beff(eb, ew16)
    use_bias0p = bool(np.abs(pb_eff[0]).max() > 1e-7)
    use_bias0e = bool(np.abs(eb_eff[0]).max() > 1e-7)

    ident = np.eye(H, dtype=np.float16)
    wp4b = np.tile(wp[None, :], (P, T)).astype(np.float32)        # [P, T*H]
    web = np.tile(we[None, :], (P, 1)).astype(np.float32)         # [P, H]

    l16, n16, l32, n32 = _blob_layout(nsub)

    def pack(blob, layout, name, arr):
        off, shape = layout[name]
        assert tuple(arr.shape) == tuple(shape), (name, arr.shape, shape)
        blob[off:off + arr.size] = arr.ravel()

    in_maps = []
    for c in range(NCORES):
        lo, hi = int(splits[c]), int(splits[c + 1])
        n_c = hi - lo
        xt5 = np.zeros((H, 5, nloc), np.float16)
        xt5[:, 0:T, :n_c] = x_proc[:, lo:hi, :].transpose(2, 0, 1).astype(np.float16)
        xt5[:, T, :n_c] = x_enc[lo:hi, :].T.astype(np.float16)
        bl = np.full(nloc, -1.0, np.float32)
        bl[:n_c] = (batch[lo:hi] - c * SEG_PER_CORE).astype(np.float32)
        blj = bl.reshape(nsub, P)
        oh = (blj[:, :, None] == np.arange(P, dtype=np.float32)[None, None, :])
        s16d = np.ascontiguousarray(
            oh.transpose(1, 0, 2).reshape(P, nsub * P)).astype(np.float16)
        cnt = np.zeros(SEG_PER_CORE, np.float64)
        segs, counts = np.unique(batch[lo:hi], return_counts=True)
        cnt[(segs - c * SEG_PER_CORE).astype(int)] = counts
        kv = (bp + be - LN2 * cnt * (wp.sum() + we.sum())).astype(np.float32)
        blob16 = np.zeros(n16, np.float16)
        pack(blob16, l16, "xt5", xt5)
        pack(blob16, l16, "s16d", s16d)
        pack(blob16, l16, "pw16", pw16)
        pack(blob16, l16, "ew16", ew16)
        pack(blob16, l16, "pb16", pb_eff.astype(np.float16))
        pack(blob16, l16, "eb16", eb_eff.astype(np.float16))
        pack(blob16, l16, "ident", ident)
        blob32 = np.zeros(n32, np.float32)
        pack(blob32, l32, "wp4b", wp4b)
        pack(blob32, l32, "web", web)
        pack(blob32, l32, "kvec", kv.reshape(P, 1).astype(np.float32))
        in_maps.append({"b16": blob16, "b32": blob32})
    meta = (nsub, use_bias0p, use_bias0e)
    return in_maps, meta


def get_runner(meta):
    key = meta
    if key not in _cache:
        nc = _build(*meta)
        _cache[key] = _Runner(nc, NCORES)
    return _cache[key]


def kernel(**inputs) -> np.ndarray:
    in_maps, meta = _prep_inputs(**inputs)
    runner = get_runner(meta)
    handle = runner.prepare(in_maps)
    outs = runner.run(handle)
    per_core = runner.results(outs)
    out = np.zeros((T, G), np.float32)
    for c in range(NCORES):
        out[:, c * SEG_PER_CORE:(c + 1) * SEG_PER_CORE] = per_core[c]["res"].T
    return out
